# revision 7
# baseline (speedup 1.0000x reference)
"""Trainium2 Bass kernel for nn_DynamicImagePrimalDualNN.

T=128 primal-dual iterations over (2,1,160,160,32) with circular FD stencils.

Distribution: mb(2) x x-slabs(4) = 8 cores (ranks 0-3 = image 0, 4-7 = image
1; slab = rank%4). y and t stay core-local.

One AllGather per iteration exchanges the two xbar edge planes; the received
halos are combined (one-hot mask multiply + parity pair-sum, all on GpSimd)
straight into the xbar tile's halo x-slots, so every compute op - including
the PE qx chain - is edge-case free.

Per-core layout: partitions p = (y%4)*32 + t (all 128 used);
free = (x_slot, yb). y/t stencils run on the TensorEngine via exact +-1
stationaries fused with -I (circular yb handled by one pad column, circular
t inside the stationary); x stencils are PE reads of shifted xbar windows.

Engine split per iteration (all tensors bf16 except x0/x1):
  PE     : u_c = q_c + grad_c(xbar) for c in {x,y,t} (PSUM accumulate),
           psX = mt' + div(q')     (14 bf16 passes, 56 chunk matmuls)
  ACT    : PSUM -> SBUF bf16 copies of u_c, pad-column maintenance
  GpSimd : halo combine, d = xbar - xn, s = mt + d
  DVE    : mt' = a*s (one 4x tensor_scalar), 6 clip passes,
           x1/xbar' (2 chunked STT sweeps reading psX + x0)

Rescaled state so every scalar is an fp32 immediate:
  mt = p/sig,  Q = q/sig,  x0 raw.
  mt' = a*(mt + xbar - xn)           (a = 1/(1+sig))
  Q'  = clip(Q + grad(xbar), lam/sig)
  psX = mt' + div(Q');  x1 = x0 - c2*psX;  xbar' = x0 - c2*(1+th)*psX
                                     (c2 = ta*sig)
"""

import math
from contextlib import ExitStack
from functools import lru_cache

import numpy as np

import concourse.bass as bass
import concourse.tile as tile
from concourse import bacc, mybir
from concourse.bass_utils import run_bass_kernel_spmd

F32 = mybir.dt.float32
BF = mybir.dt.bfloat16
AX = mybir.AluOpType
ACTF = mybir.ActivationFunctionType

T_ITERS = 128
TRACE = False
_LAST_RESULTS = None
NXS = 40          # x-slab width per core
NYB = 40          # y blocks (y = 4*yb + my)
NCH = 10          # x-chunk width for PSUM-bank-sized matmuls
GROUPS = [[0, 1, 2, 3], [4, 5, 6, 7]]
QXCH = [(0, 10), (10, 10), (20, 10), (30, 11)]   # qx-slab chunking (41 wide)


def _pidx(m, t):
    return m * 32 + t


def _stationaries():
    """(128,128) matrices W[p_in, p_out]; matmul computes out[i] = sum_k W[k,i] in[k]."""
    I = np.eye(128, dtype=np.float32)
    dy = -np.eye(128, dtype=np.float32)
    cy = np.zeros((128, 128), np.float32)
    dt = -np.eye(128, dtype=np.float32)
    dyh = -np.eye(128, dtype=np.float32)
    cyh = np.zeros((128, 128), np.float32)
    dth = -np.eye(128, dtype=np.float32)
    for t in range(32):
        for m in range(3):
            dy[_pidx(m + 1, t), _pidx(m, t)] += 1.0
        cy[_pidx(0, t), _pidx(3, t)] = 1.0
        for m in range(1, 4):
            dyh[_pidx(m - 1, t), _pidx(m, t)] += 1.0
        cyh[_pidx(3, t), _pidx(0, t)] = 1.0
        for m in range(4):
            dt[_pidx(m, (t + 1) % 32), _pidx(m, t)] += 1.0
            dth[_pidx(m, (t - 1) % 32), _pidx(m, t)] += 1.0
    return dict(w_i=I, w_ni=-I, w_dy=dy, w_cy=cy, w_dt=dt, w_dyh=dyh,
                w_cyh=cyh, w_dth=dth)


def to_dev(v):
    """(xs, 160y, 32t) -> (128, xs, 40yb) with p=(y%4)*32+t."""
    xs = v.shape[0]
    return np.ascontiguousarray(
        v.reshape(xs, NYB, 4, 32).transpose(2, 3, 0, 1).reshape(128, xs, NYB))


def from_dev(v):
    """(128, xs, 40yb) -> (xs, 160y, 32t)."""
    xs = v.shape[1]
    return np.ascontiguousarray(
        v.reshape(4, 32, xs, NYB).transpose(2, 3, 0, 1).reshape(xs, 160, 32))


def _build_nc(scalars, T=T_ITERS):
    a_, c2, th = scalars
    nc = bacc.Bacc("TRN2", target_bir_lowering=False, debug=False,
                   num_devices=8)

    dp = {}
    dp["xb0"] = nc.dram_tensor("xb0", [128, NXS, NYB], BF,
                               kind="ExternalInput")
    dp["x00"] = nc.dram_tensor("x00", [128, NXS, NYB], F32,
                               kind="ExternalInput")
    for name in ("mt0", "xn0"):
        dp[name] = nc.dram_tensor(name, [128, NXS, NYB], BF,
                                  kind="ExternalInput")
    # x-channel lambda covers the 41-wide overlap slab
    for name in ("lamx", "nlamx"):
        dp[name] = nc.dram_tensor(name, [128, NXS + 1, NYB], BF,
                                  kind="ExternalInput")
    for name in ("lamy", "nlamy", "lamt", "nlamt"):
        dp[name] = nc.dram_tensor(name, [128, NXS, NYB], BF,
                                  kind="ExternalInput")
    # one-hot over gathered slots (slot = rank_in_group*2 + e); even slots
    # select the hi halo, odd slots the lo halo (parity-separated)
    dp["msk8"] = nc.dram_tensor("msk8", [128, 8, NYB], BF,
                                kind="ExternalInput")
    wnames = list(_stationaries().keys())
    for name in wnames:
        dp[name] = nc.dram_tensor(name, [128, 128], BF, kind="ExternalInput")
    out_dram = nc.dram_tensor("out", [128, NXS, NYB], F32,
                              kind="ExternalOutput")

    with tile.TileContext(nc) as tc, ExitStack() as es:
        state = es.enter_context(tc.tile_pool(name="state", bufs=1))
        xpool = es.enter_context(tc.tile_pool(name="xp", bufs=2))
        upool = es.enter_context(tc.tile_pool(name="up", bufs=2))
        dpool = es.enter_context(tc.tile_pool(name="dram", bufs=2,
                                              space="DRAM"))
        gpool = es.enter_context(tc.tile_pool(name="gath", bufs=2))
        psum = es.enter_context(
            tc.tile_pool(name="psum", bufs=4, space=bass.MemorySpace.PSUM))
        psx_pool = es.enter_context(
            tc.tile_pool(name="psx", bufs=1, space=bass.MemorySpace.PSUM))

        # xbar: x slots 0=halo_lo, 1..40 real, 41=halo_hi; yb col 40 =
        # pad(yb0), col 41 unused (even stride keeps bf16 2x alignment)
        xbar = state.tile([128, NXS + 2, NYB + 2], BF, tag="xbar")
        # qx on the 41-wide overlap slab (col j = global x s-1+j), no halos
        qx = state.tile([128, NXS + 1, NYB], BF, tag="qx")
        # qy: yb col 0 = pad(yb39), real yb at cols 1..40, col 41 unused
        qy = state.tile([128, NXS, NYB + 2], BF, tag="qy")
        qt = state.tile([128, NXS, NYB], BF, tag="qt")
        mt = state.tile([128, NXS, NYB], BF, tag="mt")
        xn0 = state.tile([128, NXS, NYB], BF, tag="xn0")
        lamx = state.tile([128, NXS + 1, NYB], BF, tag="lamx")
        nlamx = state.tile([128, NXS + 1, NYB], BF, tag="nlamx")
        lamy = state.tile([128, NXS, NYB], BF, tag="lamy")
        nlamy = state.tile([128, NXS, NYB], BF, tag="nlamy")
        lamt = state.tile([128, NXS, NYB], BF, tag="lamt")
        nlamt = state.tile([128, NXS, NYB], BF, tag="nlamt")
        msk8 = state.tile([128, 8, NYB], BF, tag="msk8")
        W = {n: state.tile([128, 128], BF, tag=n, name=f"w_{n}")
             for n in wnames}

        nc.sync.dma_start(xbar[:, 1:41, 0:40], dp["xb0"][:])
        x0 = xpool.tile([128, NXS, NYB], F32, tag="x")
        nc.sync.dma_start(x0[:], dp["x00"][:])
        nc.sync.dma_start(mt[:], dp["mt0"][:])
        nc.sync.dma_start(xn0[:], dp["xn0"][:])
        for nm, tl in (("lamx", lamx), ("nlamx", nlamx), ("lamy", lamy),
                       ("nlamy", nlamy), ("lamt", lamt), ("nlamt", nlamt),
                       ("msk8", msk8)):
            nc.sync.dma_start(tl[:], dp[nm][:])
        for n in wnames:
            nc.sync.dma_start(W[n][:], dp[n][:])
        nc.vector.memset(qx[:], 0.0)
        nc.vector.memset(qy[:], 0.0)
        nc.vector.memset(qt[:], 0.0)
        nc.vector.tensor_copy(xbar[:, 1:41, 40:41], xbar[:, 1:41, 0:1])

        def exchange(round_idx):
            """AG of my (first,last) xbar planes; returns gathered dram tile."""
            bin_ = dpool.tile([2, 128, NYB], BF, tag="bin",
                              name=f"bin{round_idx}")
            bout = dpool.tile([8, 128, NYB], BF, tag="bout",
                              name=f"bout{round_idx}")
            nc.sync.dma_start(bin_[0], xbar[:, 1, 0:40])
            nc.sync.dma_start(bin_[1], xbar[:, 40, 0:40])
            nc.gpsimd.collective_compute(
                "AllGather", AX.bypass, replica_groups=GROUPS,
                ins=[bin_[:]], outs=[bout[:]])
            return bout

        def recv(bout):
            """Gathered planes -> SBUF; one-hot combine into xbar halo slots.

            Pure tensor_tensor ops on GpSimd so the DVE FIFO never blocks on
            the collective. hi = sum of even slots of gath*msk8, lo = odd."""
            gath = gpool.tile([128, 8, NYB], BF, tag="gath")
            nc.sync.dma_start(gath[:], bout[:].transpose([1, 0, 2]))
            tmp = gpool.tile([128, 8, NYB], BF, tag="gtmp")
            nc.gpsimd.tensor_tensor(tmp[:], gath[:], msk8[:], AX.mult)
            a4 = gpool.tile([128, 4, NYB], BF, tag="ga4")
            nc.gpsimd.tensor_tensor(a4[:], tmp[:, 0:4, :], tmp[:, 4:8, :],
                                    AX.add)
            nc.gpsimd.tensor_tensor(xbar[:, 41:42, 0:40], a4[:, 0:1, :],
                                    a4[:, 2:3, :], AX.add)
            nc.gpsimd.tensor_tensor(xbar[:, 0:1, 0:40], a4[:, 1:2, :],
                                    a4[:, 3:4, :], AX.add)

        bout = exchange(0)

        for k in range(T):
            # --- p-phase front half on GpSimd: d = xbar - xn, s = mt + d ---
            d = upool.tile([128, NXS, NYB], BF, tag="d")
            s = upool.tile([128, NXS, NYB], BF, tag="s")
            nc.gpsimd.tensor_tensor(d[:], xbar[:, 1:41, 0:40], xn0[:],
                                    AX.subtract)
            recv(bout)
            nc.gpsimd.tensor_tensor(s[:], mt[:], d[:], AX.add)

            # --- grad psums on PE; ACT copies; DVE clips ---
            uy = upool.tile([128, NXS, NYB], BF, tag="uy")
            ut = upool.tile([128, NXS, NYB], BF, tag="ut")
            ux = upool.tile([128, NXS + 1, NYB], BF, tag="ux")
            for c in range(4):
                sl = slice(1 + NCH * c, 1 + NCH * (c + 1))
                slq = slice(NCH * c, NCH * (c + 1))
                ps = psum.tile([128, NCH, NYB], F32, tag="ps")
                nc.tensor.matmul(ps[:], W["w_i"][:], qy[:, slq, 1:41],
                                 start=True, stop=False)
                nc.tensor.matmul(ps[:], W["w_dy"][:], xbar[:, sl, 0:40],
                                 start=False, stop=False)
                nc.tensor.matmul(ps[:], W["w_cy"][:], xbar[:, sl, 1:41],
                                 start=False, stop=True)
                nc.scalar.activation(uy[:, slq, :], ps[:], ACTF.Copy)
            for c in range(4):
                sl = slice(1 + NCH * c, 1 + NCH * (c + 1))
                slq = slice(NCH * c, NCH * (c + 1))
                ps = psum.tile([128, NCH, NYB], F32, tag="ps")
                nc.tensor.matmul(ps[:], W["w_i"][:], qt[:, slq, :],
                                 start=True, stop=False)
                nc.tensor.matmul(ps[:], W["w_dt"][:], xbar[:, sl, 0:40],
                                 start=False, stop=True)
                nc.scalar.activation(ut[:, slq, :], ps[:], ACTF.Copy)
            # qy clips in halves (overlap with qt/qx psum production)
            for h in (slice(0, 20), slice(20, 40)):
                nc.vector.tensor_tensor(qy[:, h, 1:41], uy[:, h, :],
                                        nlamy[:, h, :], AX.max)
                nc.vector.tensor_tensor(qy[:, h, 1:41], qy[:, h, 1:41],
                                        lamy[:, h, :], AX.min)
            # mt' = a*s: single 4x-mode tensor_scalar; s (GpSimd) is ready by
            # the time the DVE drains the qy clips
            nc.vector.tensor_scalar(mt[:], s[:], a_, None, AX.mult)
            for h in (slice(0, 20), slice(20, 40)):
                nc.vector.tensor_tensor(qt[:, h, :], ut[:, h, :],
                                        nlamt[:, h, :], AX.max)
                nc.vector.tensor_tensor(qt[:, h, :], qt[:, h, :],
                                        lamt[:, h, :], AX.min)
            # qx chain on PE (halo slots of xbar written by recv)
            for b0, w in QXCH:
                ps = psum.tile([128, w, NYB], F32, tag="ps",
                               name=f"psqx{w}")
                nc.tensor.matmul(ps[:], W["w_i"][:], qx[:, b0:b0 + w, :],
                                 start=True, stop=False)
                nc.tensor.matmul(ps[:], W["w_i"][:],
                                 xbar[:, b0 + 1:b0 + w + 1, 0:40],
                                 start=False, stop=False)
                nc.tensor.matmul(ps[:], W["w_ni"][:], xbar[:, b0:b0 + w, 0:40],
                                 start=False, stop=True)
                nc.scalar.activation(ux[:, b0:b0 + w, :], ps[:], ACTF.Copy)
            for h in (slice(0, 21), slice(21, 41)):
                nc.vector.tensor_tensor(qx[:, h, :], ux[:, h, :],
                                        nlamx[:, h, :], AX.max)
                nc.vector.tensor_tensor(qx[:, h, :], qx[:, h, :],
                                        lamx[:, h, :], AX.min)
            # qy wrap-pad (needed only by psX's w_cyh matmul); emitted after
            # the ux copies so it never head-blocks the ACT FIFO
            nc.scalar.copy(qy[:, :, 0:1], qy[:, :, 40:41])

            # --- psX = mt' + div(q') on PE (4-bank psum tile); the qx
            # terms go last in each accumulation group so chunks can start
            # before the qx clip lands ---
            psX = psx_pool.tile([128, 4, 512], F32, tag="psX")
            wins = []
            for c in (0, 3, 1, 2):
                slq = slice(NCH * c, NCH * (c + 1))          # qx[x-1]
                slq1 = slice(NCH * c + 1, NCH * (c + 1) + 1)  # qx[x]
                win = psX[:, c, 0:400].rearrange(
                    "p (x y) -> p x y", x=NCH)
                wins.append((c, win))
                nc.tensor.matmul(win, W["w_i"][:], mt[:, slq, :],
                                 start=True, stop=False)
                nc.tensor.matmul(win, W["w_dyh"][:], qy[:, slq, 1:41],
                                 start=False, stop=False)
                nc.tensor.matmul(win, W["w_cyh"][:], qy[:, slq, 0:40],
                                 start=False, stop=False)
                nc.tensor.matmul(win, W["w_dth"][:], qt[:, slq, :],
                                 start=False, stop=False)
                nc.tensor.matmul(win, W["w_i"][:], qx[:, slq, :],
                                 start=False, stop=False)
                nc.tensor.matmul(win, W["w_ni"][:], qx[:, slq1, :],
                                 start=False, stop=True)

            # --- F-phase, chunked so PE/DVE pipeline across iterations:
            # x1 = x0 - c2*psX, xbar' = x0 - c2*(1+th)*psX ---
            x1 = xpool.tile([128, NXS, NYB], F32, tag="x")
            cth = -c2 * (1.0 + th)
            if k < T - 1:
                for c, win in wins:
                    sx = slice(NCH * c, NCH * (c + 1))
                    nc.vector.scalar_tensor_tensor(
                        xbar[:, 1 + NCH * c:1 + NCH * (c + 1), 0:40], win,
                        cth, x0[:, sx, :], AX.mult, AX.add)
                    if c == 3:
                        bout = exchange(k + 1)
                    # chunk-wise pad col so next iter's w_cy matmul can start
                    nc.scalar.copy(xbar[:, 1 + NCH * c:1 + NCH * (c + 1),
                                        40:41],
                                   xbar[:, 1 + NCH * c:1 + NCH * (c + 1),
                                        0:1])
            for c, win in wins:
                sx = slice(NCH * c, NCH * (c + 1))
                nc.vector.scalar_tensor_tensor(
                    x1[:, sx, :], win, -c2, x0[:, sx, :], AX.mult, AX.add)
            x0 = x1

        nc.sync.dma_start(out_dram[:], x0[:])

    nc.compile()
    return nc


@lru_cache(maxsize=4)
def _compiled(scalars, T):
    return _build_nc(scalars, T)


def _make_in_maps(x, lambda_map, scalars, sig):
    import ml_dtypes
    bf = ml_dtypes.bfloat16
    stats = _stationaries()
    a_, c2, th = scalars
    in_maps = []
    for rank in range(8):
        mbi, pos = rank // 4, rank % 4
        s = pos * NXS
        xs = slice(s, s + NXS)
        xn = np.ascontiguousarray(x[mbi, 0, xs]).astype(np.float32)
        lam = lambda_map[mbi].astype(np.float32) / np.float32(sig)
        # x-channel lambda on the 41-wide overlap slab [s-1, s+40)
        idx = [(s - 1 + j) % 160 for j in range(NXS + 1)]
        lx = lam[0][idx]
        nxt, prv = (pos + 1) % 4, (pos - 1) % 4
        m8 = np.zeros((128, 8, NYB), np.float32)
        m8[:, 2 * nxt, :] = 1.0        # next's first plane -> halo_hi (even)
        m8[:, 2 * prv + 1, :] = 1.0    # prev's last plane  -> halo_lo (odd)
        m = dict(
            xb0=to_dev(xn).astype(bf),
            x00=to_dev(xn),
            mt0=to_dev(xn / np.float32(sig)).astype(bf),
            xn0=to_dev(xn).astype(bf),
            lamx=to_dev(lx).astype(bf), nlamx=to_dev(-lx).astype(bf),
            lamy=to_dev(lam[1][xs]).astype(bf),
            nlamy=to_dev(-lam[1][xs]).astype(bf),
            lamt=to_dev(lam[2][xs]).astype(bf),
            nlamt=to_dev(-lam[2][xs]).astype(bf),
            msk8=m8.astype(bf),
        )
        m.update({k: v.astype(bf) for k, v in stats.items()})
        in_maps.append(m)
    return in_maps


def kernel(x, lambda_map, tau, sigma, theta):
    x = np.asarray(x, dtype=np.float32)
    lambda_map = np.asarray(lambda_map, dtype=np.float32)
    L = math.sqrt(13.0)
    sig = float(1.0 / (1.0 + math.exp(-float(np.asarray(sigma)[0])))) / L
    ta = float(1.0 / (1.0 + math.exp(-float(np.asarray(tau)[0])))) / L
    th = float(1.0 / (1.0 + math.exp(-float(np.asarray(theta)[0]))))
    a_ = 1.0 / (1.0 + sig)
    c2 = ta * sig
    scalars = tuple(float(np.float32(v)) for v in (a_, c2, th))

    nc = _compiled(scalars, T_ITERS)
    in_maps = _make_in_maps(x, lambda_map, scalars, sig)
    res = run_bass_kernel_spmd(nc, in_maps, core_ids=list(range(8)),
                               trace=TRACE)
    global _LAST_RESULTS
    _LAST_RESULTS = res

    out = np.zeros((2, 1, 160, 160, 32), np.float32)
    for rank in range(8):
        mbi, pos = rank // 4, rank % 4
        s = pos * NXS
        out[mbi, 0, s:s + NXS] = from_dev(res.results[rank]["out"])
    return out


# revision 16
# speedup vs baseline: 1.2282x; 1.2282x over previous
"""Trainium2 Bass kernel for nn_DynamicImagePrimalDualNN.

T=128 primal-dual iterations over (2,1,160,160,32) with circular FD stencils.

Distribution: mb(2) x x-slabs(4) = 8 cores (ranks 0-3 = image 0, 4-7 = image
1; slab = rank%4). y and t stay core-local.

One AllGather per iteration exchanges the two xbar edge planes; the received
halos are combined (one-hot mask multiply + parity pair-sum, all on GpSimd)
straight into the xbar tile's halo x-slots, so every compute op - including
the PE qx chain - is edge-case free.

Per-core layout: partitions p = (y%4)*32 + t (all 128 used);
free = (x_slot, yb). y/t stencils run on the TensorEngine via exact +-1
stationaries fused with -I (circular yb handled by one pad column, circular
t inside the stationary); x stencils are PE reads of shifted xbar windows.

Engine split per iteration (all tensors bf16 except x0/x1):
  PE     : u_c = q_c + grad_c(xbar) for c in {x,y,t} (PSUM accumulate),
           psX = mt' + div(q')     (14 bf16 passes, 56 chunk matmuls)
  ACT    : PSUM -> SBUF bf16 copies of u_c, pad-column maintenance
  GpSimd : halo combine, d = xbar - xn, s = mt + d
  DVE    : mt' = a*s (one 4x tensor_scalar), 6 clip passes,
           x1/xbar' (2 chunked STT sweeps reading psX + x0)

Rescaled state so every scalar is an fp32 immediate:
  mt = p/sig,  Q = q/sig,  x0 raw.
  mt' = a*(mt + xbar - xn)           (a = 1/(1+sig))
  Q'  = clip(Q + grad(xbar), lam/sig)
  psX = mt' + div(Q');  x1 = x0 - c2*psX;  xbar' = x0 - c2*(1+th)*psX
                                     (c2 = ta*sig)
"""

import math
from contextlib import ExitStack
from functools import lru_cache

import numpy as np

import concourse.bass as bass
import concourse.tile as tile
from concourse import bacc, mybir
from concourse.bass_utils import run_bass_kernel_spmd

F32 = mybir.dt.float32
BF = mybir.dt.bfloat16
AX = mybir.AluOpType
ACTF = mybir.ActivationFunctionType

T_ITERS = 128
TRACE = False
_LAST_RESULTS = None
NXS = 40          # x-slab width per core
NYB = 40          # y blocks (y = 4*yb + my)
NCH = 10          # x-chunk width for PSUM-bank-sized matmuls
GROUPS = [[0, 1, 2, 3], [4, 5, 6, 7]]
QXCH = [(0, 10), (10, 10), (20, 10), (30, 11)]   # qx-slab chunking (41 wide)


def _pidx(m, t):
    return m * 32 + t


def _stationaries():
    """(128,128) matrices W[p_in, p_out]; matmul computes out[i] = sum_k W[k,i] in[k]."""
    I = np.eye(128, dtype=np.float32)
    dy = -np.eye(128, dtype=np.float32)
    cy = np.zeros((128, 128), np.float32)
    dt = -np.eye(128, dtype=np.float32)
    dyh = -np.eye(128, dtype=np.float32)
    cyh = np.zeros((128, 128), np.float32)
    dth = -np.eye(128, dtype=np.float32)
    for t in range(32):
        for m in range(3):
            dy[_pidx(m + 1, t), _pidx(m, t)] += 1.0
        cy[_pidx(0, t), _pidx(3, t)] = 1.0
        for m in range(1, 4):
            dyh[_pidx(m - 1, t), _pidx(m, t)] += 1.0
        cyh[_pidx(3, t), _pidx(0, t)] = 1.0
        for m in range(4):
            dt[_pidx(m, (t + 1) % 32), _pidx(m, t)] += 1.0
            dth[_pidx(m, (t - 1) % 32), _pidx(m, t)] += 1.0
    return dict(w_i=I, w_ni=-I, w_dy=dy, w_cy=cy, w_dt=dt, w_dyh=dyh,
                w_cyh=cyh, w_dth=dth)


def to_dev(v):
    """(xs, 160y, 32t) -> (128, xs, 40yb) with p=(y%4)*32+t."""
    xs = v.shape[0]
    return np.ascontiguousarray(
        v.reshape(xs, NYB, 4, 32).transpose(2, 3, 0, 1).reshape(128, xs, NYB))


def from_dev(v):
    """(128, xs, 40yb) -> (xs, 160y, 32t)."""
    xs = v.shape[1]
    return np.ascontiguousarray(
        v.reshape(4, 32, xs, NYB).transpose(2, 3, 0, 1).reshape(xs, 160, 32))


def _build_nc(scalars, T=T_ITERS):
    a_, c2, th = scalars
    nc = bacc.Bacc("TRN2", target_bir_lowering=False, debug=False,
                   num_devices=8)

    dp = {}
    dp["xb0"] = nc.dram_tensor("xb0", [128, NXS, NYB], BF,
                               kind="ExternalInput")
    dp["x00"] = nc.dram_tensor("x00", [128, NXS, NYB], F32,
                               kind="ExternalInput")
    for name in ("mt0", "cxn"):
        dp[name] = nc.dram_tensor(name, [128, NXS, NYB], BF,
                                  kind="ExternalInput")
    # x-channel lambda covers the 41-wide overlap slab
    for name in ("lamx", "nlamx"):
        dp[name] = nc.dram_tensor(name, [128, NXS + 1, NYB], BF,
                                  kind="ExternalInput")
    for name in ("lamy", "nlamy", "lamt", "nlamt"):
        dp[name] = nc.dram_tensor(name, [128, NXS, NYB], BF,
                                  kind="ExternalInput")
    # (128, 8) one-hot masks over gathered slots (slot = rank_in_group*2 + e)
    for name in ("mskhi", "msklo"):
        dp[name] = nc.dram_tensor(name, [128, 8], F32, kind="ExternalInput")
    wnames = list(_stationaries().keys())
    for name in wnames:
        dp[name] = nc.dram_tensor(name, [128, 128], BF, kind="ExternalInput")
    out_dram = nc.dram_tensor("out", [128, NXS, NYB], F32,
                              kind="ExternalOutput")

    with tile.TileContext(nc) as tc, ExitStack() as es:
        state = es.enter_context(tc.tile_pool(name="state", bufs=1))
        xpool = es.enter_context(tc.tile_pool(name="xp", bufs=2))
        upool = es.enter_context(tc.tile_pool(name="up", bufs=2))
        dpool = es.enter_context(tc.tile_pool(name="dram", bufs=2,
                                              space="DRAM"))
        gpool = es.enter_context(tc.tile_pool(name="gath", bufs=2))
        psum = es.enter_context(
            tc.tile_pool(name="psum", bufs=4, space=bass.MemorySpace.PSUM))
        psx_pool = es.enter_context(
            tc.tile_pool(name="psx", bufs=1, space=bass.MemorySpace.PSUM))

        # xbar: x slots 0=halo_lo, 1..40 real, 41=halo_hi; yb col 40 =
        # pad(yb0), col 41 unused (even stride keeps bf16 2x alignment)
        xbar = state.tile([128, NXS + 2, NYB + 2], BF, tag="xbar")
        # qx on the 41-wide overlap slab (col j = global x s-1+j), no halos
        qx = state.tile([128, NXS + 1, NYB], BF, tag="qx")
        # qy: yb col 0 = pad(yb39), real yb at cols 1..40, col 41 unused
        qy = state.tile([128, NXS, NYB + 2], BF, tag="qy")
        qt = state.tile([128, NXS, NYB], BF, tag="qt")
        mt = state.tile([128, NXS, NYB], BF, tag="mt")
        cxn = state.tile([128, NXS, NYB], BF, tag="cxn")
        lamx = state.tile([128, NXS + 1, NYB], BF, tag="lamx")
        nlamx = state.tile([128, NXS + 1, NYB], BF, tag="nlamx")
        lamy = state.tile([128, NXS, NYB], BF, tag="lamy")
        nlamy = state.tile([128, NXS, NYB], BF, tag="nlamy")
        lamt = state.tile([128, NXS, NYB], BF, tag="lamt")
        nlamt = state.tile([128, NXS, NYB], BF, tag="nlamt")
        mskhi = state.tile([128, 8], F32, tag="mskhi")
        msklo = state.tile([128, 8], F32, tag="msklo")
        W = {n: state.tile([128, 128], BF, tag=n, name=f"w_{n}")
             for n in wnames}

        nc.sync.dma_start(xbar[:, 1:41, 0:40], dp["xb0"][:])
        x0 = xpool.tile([128, NXS, NYB], F32, tag="x")
        nc.sync.dma_start(x0[:], dp["x00"][:])
        nc.sync.dma_start(mt[:], dp["mt0"][:])
        nc.sync.dma_start(cxn[:], dp["cxn"][:])
        for nm, tl in (("lamx", lamx), ("nlamx", nlamx), ("lamy", lamy),
                       ("nlamy", nlamy), ("lamt", lamt), ("nlamt", nlamt),
                       ("mskhi", mskhi), ("msklo", msklo)):
            nc.sync.dma_start(tl[:], dp[nm][:])
        for n in wnames:
            nc.sync.dma_start(W[n][:], dp[n][:])
        nc.vector.memset(qx[:], 0.0)
        nc.vector.memset(qy[:], 0.0)
        nc.vector.memset(qt[:], 0.0)
        nc.vector.tensor_copy(xbar[:, 1:41, 40:41], xbar[:, 1:41, 0:1])

        def exchange(round_idx):
            """AG of my (first,last) xbar planes; returns gathered dram tile."""
            bin_ = dpool.tile([2, 128, NYB], BF, tag="bin",
                              name=f"bin{round_idx}")
            bout = dpool.tile([8, 128, NYB], BF, tag="bout",
                              name=f"bout{round_idx}")
            nc.sync.dma_start(bin_[0], xbar[:, 1, 0:40])
            nc.sync.dma_start(bin_[1], xbar[:, 40, 0:40])
            nc.gpsimd.collective_compute(
                "AllGather", AX.bypass, replica_groups=GROUPS,
                ins=[bin_[:]], outs=[bout[:]])
            return bout

        def recv(bout):
            """Gathered planes -> SBUF; mask-combine into xbar halo slots.

            DVE ops, but emitted mid-iteration (after the qy clips) so the
            DVE FIFO never head-blocks on the collective."""
            gath = gpool.tile([128, 8, NYB], BF, tag="gath")
            nc.sync.dma_start(gath[:], bout[:].transpose([1, 0, 2]))
            hi = xbar[:, 41, 0:40]
            lo = xbar[:, 0, 0:40]
            nc.vector.tensor_scalar(hi, gath[:, 0, :], mskhi[:, 0:1],
                                    None, AX.mult)
            nc.vector.tensor_scalar(lo, gath[:, 1, :], msklo[:, 1:2],
                                    None, AX.mult)
            for j in (1, 2, 3):
                nc.vector.scalar_tensor_tensor(
                    hi, gath[:, 2 * j, :], mskhi[:, 2 * j:2 * j + 1],
                    hi, AX.mult, AX.add)
                nc.vector.scalar_tensor_tensor(
                    lo, gath[:, 2 * j + 1, :],
                    msklo[:, 2 * j + 1:2 * j + 2], lo, AX.mult, AX.add)

        bout = exchange(0)

        for k in range(T):
            # --- p-phase: t1 = a*xbar - cxn (ready at iter start) ---
            t1 = upool.tile([128, NXS, NYB], BF, tag="t1")
            nc.vector.scalar_tensor_tensor(t1[:], xbar[:, 1:41, 0:40], a_,
                                           cxn[:], AX.mult, AX.subtract)

            # --- grad psums on PE; ACT copies; DVE clips ---
            uy = upool.tile([128, NXS, NYB], BF, tag="uy")
            ut = upool.tile([128, NXS, NYB], BF, tag="ut")
            ux = upool.tile([128, NXS + 1, NYB], BF, tag="ux")
            for c in range(4):
                sl = slice(1 + NCH * c, 1 + NCH * (c + 1))
                slq = slice(NCH * c, NCH * (c + 1))
                ps = psum.tile([128, NCH, NYB], F32, tag="ps")
                nc.tensor.matmul(ps[:], W["w_i"][:], qy[:, slq, 1:41],
                                 start=True, stop=False)
                nc.tensor.matmul(ps[:], W["w_dy"][:], xbar[:, sl, 0:40],
                                 start=False, stop=False)
                nc.tensor.matmul(ps[:], W["w_cy"][:], xbar[:, sl, 1:41],
                                 start=False, stop=True)
                nc.scalar.activation(uy[:, slq, :], ps[:], ACTF.Copy)
            for c in range(4):
                sl = slice(1 + NCH * c, 1 + NCH * (c + 1))
                slq = slice(NCH * c, NCH * (c + 1))
                ps = psum.tile([128, NCH, NYB], F32, tag="ps")
                nc.tensor.matmul(ps[:], W["w_i"][:], qt[:, slq, :],
                                 start=True, stop=False)
                nc.tensor.matmul(ps[:], W["w_dt"][:], xbar[:, sl, 0:40],
                                 start=False, stop=True)
                nc.scalar.activation(ut[:, slq, :], ps[:], ACTF.Copy)
            # qy clips in halves (overlap with qt/qx psum production)
            for h in (slice(0, 20), slice(20, 40)):
                nc.vector.tensor_tensor(qy[:, h, 1:41], uy[:, h, :],
                                        nlamy[:, h, :], AX.max)
                nc.vector.tensor_tensor(qy[:, h, 1:41], qy[:, h, 1:41],
                                        lamy[:, h, :], AX.min)
            # mt' = a*mt + t1 (t1 done long before the qy clips drain)
            nc.vector.scalar_tensor_tensor(mt[:], mt[:], a_, t1[:],
                                           AX.mult, AX.add)
            # halo combine here: the gather DMA has landed by now, so no
            # FIFO head-block; only the PE qx chain consumes the halos
            recv(bout)
            for h in (slice(0, 20), slice(20, 40)):
                nc.vector.tensor_tensor(qt[:, h, :], ut[:, h, :],
                                        nlamt[:, h, :], AX.max)
                nc.vector.tensor_tensor(qt[:, h, :], qt[:, h, :],
                                        lamt[:, h, :], AX.min)
            # qx chain on PE (halo slots of xbar written by recv)
            for b0, w in QXCH:
                ps = psum.tile([128, w, NYB], F32, tag="ps",
                               name=f"psqx{w}")
                nc.tensor.matmul(ps[:], W["w_i"][:], qx[:, b0:b0 + w, :],
                                 start=True, stop=False)
                nc.tensor.matmul(ps[:], W["w_i"][:],
                                 xbar[:, b0 + 1:b0 + w + 1, 0:40],
                                 start=False, stop=False)
                nc.tensor.matmul(ps[:], W["w_ni"][:], xbar[:, b0:b0 + w, 0:40],
                                 start=False, stop=True)
                nc.scalar.activation(ux[:, b0:b0 + w, :], ps[:], ACTF.Copy)
            for h in (slice(0, 21), slice(21, 41)):
                nc.vector.tensor_tensor(qx[:, h, :], ux[:, h, :],
                                        nlamx[:, h, :], AX.max)
                nc.vector.tensor_tensor(qx[:, h, :], qx[:, h, :],
                                        lamx[:, h, :], AX.min)
            # qy wrap-pad (needed only by psX's w_cyh matmul); emitted after
            # the ux copies so it never head-blocks the ACT FIFO
            nc.scalar.copy(qy[:, :, 0:1], qy[:, :, 40:41])

            # --- psX = mt' + div(q') on PE (4-bank psum tile); the qx
            # terms go last in each accumulation group so chunks can start
            # before the qx clip lands ---
            psX = psx_pool.tile([128, 4, 512], F32, tag="psX")
            wins = []
            for c in (0, 3, 1, 2):
                slq = slice(NCH * c, NCH * (c + 1))          # qx[x-1]
                slq1 = slice(NCH * c + 1, NCH * (c + 1) + 1)  # qx[x]
                win = psX[:, c, 0:400].rearrange(
                    "p (x y) -> p x y", x=NCH)
                wins.append((c, win))
                nc.tensor.matmul(win, W["w_i"][:], mt[:, slq, :],
                                 start=True, stop=False)
                nc.tensor.matmul(win, W["w_dyh"][:], qy[:, slq, 1:41],
                                 start=False, stop=False)
                nc.tensor.matmul(win, W["w_cyh"][:], qy[:, slq, 0:40],
                                 start=False, stop=False)
                nc.tensor.matmul(win, W["w_dth"][:], qt[:, slq, :],
                                 start=False, stop=False)
                nc.tensor.matmul(win, W["w_i"][:], qx[:, slq, :],
                                 start=False, stop=False)
                nc.tensor.matmul(win, W["w_ni"][:], qx[:, slq1, :],
                                 start=False, stop=True)

            # --- F-phase, chunked so PE/DVE pipeline across iterations:
            # x1 = x0 - c2*psX, xbar' = x0 - c2*(1+th)*psX ---
            x1 = xpool.tile([128, NXS, NYB], F32, tag="x")
            cth = -c2 * (1.0 + th)
            if k < T - 1:
                for c, win in wins:
                    sx = slice(NCH * c, NCH * (c + 1))
                    nc.vector.scalar_tensor_tensor(
                        xbar[:, 1 + NCH * c:1 + NCH * (c + 1), 0:40], win,
                        cth, x0[:, sx, :], AX.mult, AX.add)
                    if c == 3:
                        bout = exchange(k + 1)
                    # chunk-wise pad col so next iter's w_cy matmul can start
                    nc.scalar.copy(xbar[:, 1 + NCH * c:1 + NCH * (c + 1),
                                        40:41],
                                   xbar[:, 1 + NCH * c:1 + NCH * (c + 1),
                                        0:1])
            for c, win in wins:
                sx = slice(NCH * c, NCH * (c + 1))
                nc.vector.scalar_tensor_tensor(
                    x1[:, sx, :], win, -c2, x0[:, sx, :], AX.mult, AX.add)
            x0 = x1

        nc.sync.dma_start(out_dram[:], x0[:])

    nc.compile()
    return nc


@lru_cache(maxsize=4)
def _compiled(scalars, T):
    return _build_nc(scalars, T)


def _make_in_maps(x, lambda_map, scalars, sig):
    import ml_dtypes
    bf = ml_dtypes.bfloat16
    stats = _stationaries()
    a_, c2, th = scalars
    in_maps = []
    for rank in range(8):
        mbi, pos = rank // 4, rank % 4
        s = pos * NXS
        xs = slice(s, s + NXS)
        xn = np.ascontiguousarray(x[mbi, 0, xs]).astype(np.float32)
        lam = lambda_map[mbi].astype(np.float32) / np.float32(sig)
        # x-channel lambda on the 41-wide overlap slab [s-1, s+40)
        idx = [(s - 1 + j) % 160 for j in range(NXS + 1)]
        lx = lam[0][idx]
        nxt, prv = (pos + 1) % 4, (pos - 1) % 4
        mhi = np.zeros((128, 8), np.float32)
        mlo = np.zeros((128, 8), np.float32)
        mhi[:, 2 * nxt] = 1.0        # next's first plane -> halo_hi
        mlo[:, 2 * prv + 1] = 1.0    # prev's last plane  -> halo_lo
        m = dict(
            xb0=to_dev(xn).astype(bf),
            x00=to_dev(xn),
            mt0=to_dev(xn / np.float32(sig)).astype(bf),
            cxn=to_dev(np.float32(a_) * xn).astype(bf),
            lamx=to_dev(lx).astype(bf), nlamx=to_dev(-lx).astype(bf),
            lamy=to_dev(lam[1][xs]).astype(bf),
            nlamy=to_dev(-lam[1][xs]).astype(bf),
            lamt=to_dev(lam[2][xs]).astype(bf),
            nlamt=to_dev(-lam[2][xs]).astype(bf),
            mskhi=mhi, msklo=mlo,
        )
        m.update({k: v.astype(bf) for k, v in stats.items()})
        in_maps.append(m)
    return in_maps


def kernel(x, lambda_map, tau, sigma, theta):
    x = np.asarray(x, dtype=np.float32)
    lambda_map = np.asarray(lambda_map, dtype=np.float32)
    L = math.sqrt(13.0)
    sig = float(1.0 / (1.0 + math.exp(-float(np.asarray(sigma)[0])))) / L
    ta = float(1.0 / (1.0 + math.exp(-float(np.asarray(tau)[0])))) / L
    th = float(1.0 / (1.0 + math.exp(-float(np.asarray(theta)[0]))))
    a_ = 1.0 / (1.0 + sig)
    c2 = ta * sig
    scalars = tuple(float(np.float32(v)) for v in (a_, c2, th))

    nc = _compiled(scalars, T_ITERS)
    in_maps = _make_in_maps(x, lambda_map, scalars, sig)
    res = run_bass_kernel_spmd(nc, in_maps, core_ids=list(range(8)),
                               trace=TRACE)
    global _LAST_RESULTS
    _LAST_RESULTS = res

    out = np.zeros((2, 1, 160, 160, 32), np.float32)
    for rank in range(8):
        mbi, pos = rank // 4, rank % 4
        s = pos * NXS
        out[mbi, 0, s:s + NXS] = from_dev(res.results[rank]["out"])
    return out


# revision 19
# speedup vs baseline: 1.2937x; 1.0533x over previous
"""Trainium2 Bass kernel for nn_DynamicImagePrimalDualNN.

T=128 primal-dual iterations over (2,1,160,160,32) with circular FD stencils.

Distribution: mb(2) x x-slabs(4) = 8 cores (ranks 0-3 = image 0, 4-7 = image
1; slab = rank%4). y and t stay core-local.

One AllGather per iteration exchanges the two xbar edge planes; the received
halos are combined (one-hot mask multiply + parity pair-sum, all on GpSimd)
straight into the xbar tile's halo x-slots, so every compute op - including
the PE qx chain - is edge-case free.

Per-core layout: partitions p = (y%4)*32 + t (all 128 used);
free = (x_slot, yb). y/t stencils run on the TensorEngine via exact +-1
stationaries fused with -I (circular yb handled by one pad column, circular
t inside the stationary); x stencils are PE reads of shifted xbar windows.

Engine split per iteration (all tensors bf16 except x0/x1):
  PE     : u_c = q_c + grad_c(xbar) for c in {x,y,t} (PSUM accumulate),
           psX = mt' + div(q')     (14 bf16 passes, 56 chunk matmuls)
  ACT    : PSUM -> SBUF bf16 copies of u_c, pad-column maintenance
  GpSimd : halo combine, d = xbar - xn, s = mt + d
  DVE    : mt' = a*s (one 4x tensor_scalar), 6 clip passes,
           x1/xbar' (2 chunked STT sweeps reading psX + x0)

Rescaled state so every scalar is an fp32 immediate:
  mt = p/sig,  Q = q/sig,  x0 raw.
  mt' = a*(mt + xbar - xn)           (a = 1/(1+sig))
  Q'  = clip(Q + grad(xbar), lam/sig)
  psX = mt' + div(Q');  x1 = x0 - c2*psX;  xbar' = x0 - c2*(1+th)*psX
                                     (c2 = ta*sig)
"""

import math
from contextlib import ExitStack
from functools import lru_cache

import numpy as np

import concourse.bass as bass
import concourse.tile as tile
from concourse import bacc, mybir
from concourse.bass_utils import run_bass_kernel_spmd

F32 = mybir.dt.float32
BF = mybir.dt.float16  # fp16: 10-bit mantissa, |state| << 65504
AX = mybir.AluOpType
ACTF = mybir.ActivationFunctionType

T_ITERS = 128
TRACE = False
_LAST_RESULTS = None
NXS = 40          # x-slab width per core
NYB = 40          # y blocks (y = 4*yb + my)
NCH = 10          # x-chunk width for PSUM-bank-sized matmuls
GROUPS = [[0, 1, 2, 3], [4, 5, 6, 7]]
QXCH = [(0, 10), (10, 10), (20, 10), (30, 11)]   # qx-slab chunking (41 wide)


def _pidx(m, t):
    return m * 32 + t


def _stationaries():
    """(128,128) matrices W[p_in, p_out]; matmul computes out[i] = sum_k W[k,i] in[k]."""
    I = np.eye(128, dtype=np.float32)
    dy = -np.eye(128, dtype=np.float32)
    cy = np.zeros((128, 128), np.float32)
    dt = -np.eye(128, dtype=np.float32)
    dyh = -np.eye(128, dtype=np.float32)
    cyh = np.zeros((128, 128), np.float32)
    dth = -np.eye(128, dtype=np.float32)
    for t in range(32):
        for m in range(3):
            dy[_pidx(m + 1, t), _pidx(m, t)] += 1.0
        cy[_pidx(0, t), _pidx(3, t)] = 1.0
        for m in range(1, 4):
            dyh[_pidx(m - 1, t), _pidx(m, t)] += 1.0
        cyh[_pidx(3, t), _pidx(0, t)] = 1.0
        for m in range(4):
            dt[_pidx(m, (t + 1) % 32), _pidx(m, t)] += 1.0
            dth[_pidx(m, (t - 1) % 32), _pidx(m, t)] += 1.0
    return dict(w_i=I, w_ni=-I, w_dy=dy, w_cy=cy, w_dt=dt, w_dyh=dyh,
                w_cyh=cyh, w_dth=dth)


def to_dev(v):
    """(xs, 160y, 32t) -> (128, xs, 40yb) with p=(y%4)*32+t."""
    xs = v.shape[0]
    return np.ascontiguousarray(
        v.reshape(xs, NYB, 4, 32).transpose(2, 3, 0, 1).reshape(128, xs, NYB))


def from_dev(v):
    """(128, xs, 40yb) -> (xs, 160y, 32t)."""
    xs = v.shape[1]
    return np.ascontiguousarray(
        v.reshape(4, 32, xs, NYB).transpose(2, 3, 0, 1).reshape(xs, 160, 32))


def _build_nc(scalars, T=T_ITERS):
    a_, c2, th = scalars
    nc = bacc.Bacc("TRN2", target_bir_lowering=False, debug=False,
                   num_devices=8)

    dp = {}
    dp["xb0"] = nc.dram_tensor("xb0", [128, NXS, NYB], BF,
                               kind="ExternalInput")
    dp["x00"] = nc.dram_tensor("x00", [128, NXS, NYB], F32,
                               kind="ExternalInput")
    for name in ("mt0", "cxn"):
        dp[name] = nc.dram_tensor(name, [128, NXS, NYB], BF,
                                  kind="ExternalInput")
    # x-channel lambda covers the 41-wide overlap slab
    for name in ("lamx", "nlamx"):
        dp[name] = nc.dram_tensor(name, [128, NXS + 1, NYB], BF,
                                  kind="ExternalInput")
    for name in ("lamy", "nlamy", "lamt", "nlamt"):
        dp[name] = nc.dram_tensor(name, [128, NXS, NYB], BF,
                                  kind="ExternalInput")
    # (128, 8) one-hot masks over gathered slots (slot = rank_in_group*2 + e)
    for name in ("mskhi", "msklo"):
        dp[name] = nc.dram_tensor(name, [128, 8], F32, kind="ExternalInput")
    wnames = list(_stationaries().keys())
    for name in wnames:
        dp[name] = nc.dram_tensor(name, [128, 128], BF, kind="ExternalInput")
    out_dram = nc.dram_tensor("out", [128, NXS, NYB], F32,
                              kind="ExternalOutput")

    with tile.TileContext(nc) as tc, ExitStack() as es:
        state = es.enter_context(tc.tile_pool(name="state", bufs=1))
        xpool = es.enter_context(tc.tile_pool(name="xp", bufs=2))
        upool = es.enter_context(tc.tile_pool(name="up", bufs=2))
        dpool = es.enter_context(tc.tile_pool(name="dram", bufs=2,
                                              space="DRAM"))
        gpool = es.enter_context(tc.tile_pool(name="gath", bufs=2))
        psum = es.enter_context(
            tc.tile_pool(name="psum", bufs=4, space=bass.MemorySpace.PSUM))
        psx_pool = es.enter_context(
            tc.tile_pool(name="psx", bufs=1, space=bass.MemorySpace.PSUM))

        # xbar: x slots 0=halo_lo, 1..40 real, 41=halo_hi; yb col 40 =
        # pad(yb0), col 41 unused (even stride keeps bf16 2x alignment)
        xbar = state.tile([128, NXS + 2, NYB + 2], BF, tag="xbar")
        # qx on the 41-wide overlap slab (col j = global x s-1+j), no halos
        qx = state.tile([128, NXS + 1, NYB], BF, tag="qx")
        # qy: yb col 0 = pad(yb39), real yb at cols 1..40, col 41 unused
        qy = state.tile([128, NXS, NYB + 2], BF, tag="qy")
        qt = state.tile([128, NXS, NYB], BF, tag="qt")
        mt = state.tile([128, NXS, NYB], BF, tag="mt")
        cxn = state.tile([128, NXS, NYB], BF, tag="cxn")
        lamx = state.tile([128, NXS + 1, NYB], BF, tag="lamx")
        nlamx = state.tile([128, NXS + 1, NYB], BF, tag="nlamx")
        lamy = state.tile([128, NXS, NYB], BF, tag="lamy")
        nlamy = state.tile([128, NXS, NYB], BF, tag="nlamy")
        lamt = state.tile([128, NXS, NYB], BF, tag="lamt")
        nlamt = state.tile([128, NXS, NYB], BF, tag="nlamt")
        mskhi = state.tile([128, 8], F32, tag="mskhi")
        msklo = state.tile([128, 8], F32, tag="msklo")
        W = {n: state.tile([128, 128], BF, tag=n, name=f"w_{n}")
             for n in wnames}

        nc.sync.dma_start(xbar[:, 1:41, 0:40], dp["xb0"][:])
        x0 = xpool.tile([128, NXS, NYB], F32, tag="x")
        nc.sync.dma_start(x0[:], dp["x00"][:])
        nc.sync.dma_start(mt[:], dp["mt0"][:])
        nc.sync.dma_start(cxn[:], dp["cxn"][:])
        for nm, tl in (("lamx", lamx), ("nlamx", nlamx), ("lamy", lamy),
                       ("nlamy", nlamy), ("lamt", lamt), ("nlamt", nlamt),
                       ("mskhi", mskhi), ("msklo", msklo)):
            nc.sync.dma_start(tl[:], dp[nm][:])
        for n in wnames:
            nc.sync.dma_start(W[n][:], dp[n][:])
        nc.vector.memset(qx[:], 0.0)
        nc.vector.memset(qy[:], 0.0)
        nc.vector.memset(qt[:], 0.0)
        nc.vector.tensor_copy(xbar[:, 1:41, 40:41], xbar[:, 1:41, 0:1])

        def exchange(round_idx):
            """AG of my (first,last) xbar planes; returns gathered dram tile."""
            bin_ = dpool.tile([2, 128, NYB], BF, tag="bin",
                              name=f"bin{round_idx}")
            bout = dpool.tile([8, 128, NYB], BF, tag="bout",
                              name=f"bout{round_idx}")
            nc.sync.dma_start(bin_[0], xbar[:, 1, 0:40])
            nc.sync.dma_start(bin_[1], xbar[:, 40, 0:40])
            nc.gpsimd.collective_compute(
                "AllGather", AX.bypass, replica_groups=GROUPS,
                ins=[bin_[:]], outs=[bout[:]])
            return bout

        def recv(bout):
            """Gathered planes -> SBUF; mask-combine into xbar halo slots.

            DVE ops, but emitted mid-iteration (after the qy clips) so the
            DVE FIFO never head-blocks on the collective."""
            gath = gpool.tile([128, 8, NYB], BF, tag="gath")
            nc.sync.dma_start(gath[:], bout[:].transpose([1, 0, 2]))
            hi = xbar[:, 41, 0:40]
            lo = xbar[:, 0, 0:40]
            nc.vector.tensor_scalar(hi, gath[:, 0, :], mskhi[:, 0:1],
                                    None, AX.mult)
            nc.vector.tensor_scalar(lo, gath[:, 1, :], msklo[:, 1:2],
                                    None, AX.mult)
            for j in (1, 2, 3):
                nc.vector.scalar_tensor_tensor(
                    hi, gath[:, 2 * j, :], mskhi[:, 2 * j:2 * j + 1],
                    hi, AX.mult, AX.add)
                nc.vector.scalar_tensor_tensor(
                    lo, gath[:, 2 * j + 1, :],
                    msklo[:, 2 * j + 1:2 * j + 2], lo, AX.mult, AX.add)

        bout = exchange(0)

        for k in range(T):
            # --- grad psums on PE; ACT copies; DVE clips ---
            uy = upool.tile([128, NXS, NYB], BF, tag="uy")
            ut = upool.tile([128, NXS, NYB], BF, tag="ut")
            ux = upool.tile([128, NXS + 1, NYB], BF, tag="ux")
            amx = upool.tile([128, NXS, NYB], BF, tag="amx")
            for c in range(4):
                sl = slice(1 + NCH * c, 1 + NCH * (c + 1))
                slq = slice(NCH * c, NCH * (c + 1))
                ps = psum.tile([128, NCH, NYB], F32, tag="ps")
                nc.tensor.matmul(ps[:], W["w_i"][:], qy[:, slq, 1:41],
                                 start=True, stop=False)
                nc.tensor.matmul(ps[:], W["w_dy"][:], xbar[:, sl, 0:40],
                                 start=False, stop=False)
                nc.tensor.matmul(ps[:], W["w_cy"][:], xbar[:, sl, 1:41],
                                 start=False, stop=True)
                nc.scalar.activation(uy[:, slq, :], ps[:], ACTF.Copy)
            for c in range(4):
                sl = slice(1 + NCH * c, 1 + NCH * (c + 1))
                slq = slice(NCH * c, NCH * (c + 1))
                ps = psum.tile([128, NCH, NYB], F32, tag="ps")
                nc.tensor.matmul(ps[:], W["w_i"][:], qt[:, slq, :],
                                 start=True, stop=False)
                nc.tensor.matmul(ps[:], W["w_dt"][:], xbar[:, sl, 0:40],
                                 start=False, stop=True)
                nc.scalar.activation(ut[:, slq, :], ps[:], ACTF.Copy)
            # p-phase front half on PE: amx = a*(mt + xbar) via PSUM
            # accumulate + ACT scale-copy (fp32 internal, one rounding)
            for c in range(4):
                sl = slice(1 + NCH * c, 1 + NCH * (c + 1))
                slq = slice(NCH * c, NCH * (c + 1))
                ps = psum.tile([128, NCH, NYB], F32, tag="ps")
                nc.tensor.matmul(ps[:], W["w_i"][:], mt[:, slq, :],
                                 start=True, stop=False)
                nc.tensor.matmul(ps[:], W["w_i"][:], xbar[:, sl, 0:40],
                                 start=False, stop=True)
                nc.scalar.activation(amx[:, slq, :], ps[:], ACTF.Copy,
                                     scale=a_)
            # qy clips in halves (overlap with qt/p/qx psum production)
            for h in (slice(0, 20), slice(20, 40)):
                nc.vector.tensor_tensor(qy[:, h, 1:41], uy[:, h, :],
                                        nlamy[:, h, :], AX.max)
                nc.vector.tensor_tensor(qy[:, h, 1:41], qy[:, h, 1:41],
                                        lamy[:, h, :], AX.min)
            # halo combine here: the gather DMA has landed by now, so no
            # FIFO head-block; only the PE qx chain consumes the halos
            recv(bout)
            for h in (slice(0, 20), slice(20, 40)):
                nc.vector.tensor_tensor(qt[:, h, :], ut[:, h, :],
                                        nlamt[:, h, :], AX.max)
                nc.vector.tensor_tensor(qt[:, h, :], qt[:, h, :],
                                        lamt[:, h, :], AX.min)
            # mt' = amx - cxn (single 2x tensor_tensor)
            nc.vector.tensor_tensor(mt[:], amx[:], cxn[:], AX.subtract)
            # qx chain on PE; halo-free chunks (1,2) first so the PE keeps
            # running while the halo combine lands
            for b0, w in (QXCH[1], QXCH[2], QXCH[0], QXCH[3]):
                ps = psum.tile([128, w, NYB], F32, tag="ps",
                               name=f"psqx{b0}_{w}")
                nc.tensor.matmul(ps[:], W["w_i"][:], qx[:, b0:b0 + w, :],
                                 start=True, stop=False)
                nc.tensor.matmul(ps[:], W["w_i"][:],
                                 xbar[:, b0 + 1:b0 + w + 1, 0:40],
                                 start=False, stop=False)
                nc.tensor.matmul(ps[:], W["w_ni"][:], xbar[:, b0:b0 + w, 0:40],
                                 start=False, stop=True)
                nc.scalar.activation(ux[:, b0:b0 + w, :], ps[:], ACTF.Copy)
            for h in (slice(10, 30), slice(0, 10), slice(30, 41)):
                nc.vector.tensor_tensor(qx[:, h, :], ux[:, h, :],
                                        nlamx[:, h, :], AX.max)
                nc.vector.tensor_tensor(qx[:, h, :], qx[:, h, :],
                                        lamx[:, h, :], AX.min)
            # qy wrap-pad (needed only by psX's w_cyh matmul); emitted after
            # the ux copies so it never head-blocks the ACT FIFO
            nc.scalar.copy(qy[:, :, 0:1], qy[:, :, 40:41])

            # --- psX = mt' + div(q') on PE (4-bank psum tile); the qx
            # terms go last in each accumulation group so chunks can start
            # before the qx clip lands ---
            psX = psx_pool.tile([128, 4, 512], F32, tag="psX")
            for c in (0, 3, 1, 2):
                slq = slice(NCH * c, NCH * (c + 1))          # qx[x-1]
                slq1 = slice(NCH * c + 1, NCH * (c + 1) + 1)  # qx[x]
                win = psX[:, c, 0:400].rearrange(
                    "p (x y) -> p x y", x=NCH)
                nc.tensor.matmul(win, W["w_dyh"][:], qy[:, slq, 1:41],
                                 start=True, stop=False)
                nc.tensor.matmul(win, W["w_cyh"][:], qy[:, slq, 0:40],
                                 start=False, stop=False)
                nc.tensor.matmul(win, W["w_dth"][:], qt[:, slq, :],
                                 start=False, stop=False)
                nc.tensor.matmul(win, W["w_i"][:], mt[:, slq, :],
                                 start=False, stop=False)
                nc.tensor.matmul(win, W["w_i"][:], qx[:, slq, :],
                                 start=False, stop=False)
                nc.tensor.matmul(win, W["w_ni"][:], qx[:, slq1, :],
                                 start=False, stop=True)

            # --- F-phase: x1 = x0 - c2*psX, xbar' = x0 - c2*(1+th)*psX.
            # Two tiny edge STTs feed the AllGather as soon as psX chunks
            # 0 and 3 stop; the bulk runs as half-sweeps so PE/DVE pipeline
            # across iterations ---
            x1 = xpool.tile([128, NXS, NYB], F32, tag="x")
            cth = -c2 * (1.0 + th)
            if k < T - 1:
                nc.vector.scalar_tensor_tensor(
                    xbar[:, 1:2, 0:40], psX[:, 0:1, 0:40], cth,
                    x0[:, 0:1, :], AX.mult, AX.add)
                nc.vector.scalar_tensor_tensor(
                    xbar[:, 40:41, 0:40], psX[:, 3:4, 360:400], cth,
                    x0[:, 39:40, :], AX.mult, AX.add)
                bout = exchange(k + 1)
                for hb in (0, 1):
                    pv = psX[:, 2 * hb:2 * hb + 2, 0:400].rearrange(
                        "p c (x y) -> p c x y", x=NCH)
                    xsl = slice(20 * hb, 20 * hb + 20)
                    nc.vector.scalar_tensor_tensor(
                        xbar[:, 1 + 20 * hb:21 + 20 * hb, 0:40].rearrange(
                            "p (c x) y -> p c x y", c=2),
                        pv, cth,
                        x0[:, xsl, :].rearrange("p (c x) y -> p c x y", c=2),
                        AX.mult, AX.add)
                    # half-wise pad col so next iter's w_cy matmul can start
                    nc.scalar.copy(
                        xbar[:, 1 + 20 * hb:21 + 20 * hb, 40:41],
                        xbar[:, 1 + 20 * hb:21 + 20 * hb, 0:1])
            for hb in (0, 1):
                pv = psX[:, 2 * hb:2 * hb + 2, 0:400].rearrange(
                    "p c (x y) -> p c x y", x=NCH)
                xsl = slice(20 * hb, 20 * hb + 20)
                nc.vector.scalar_tensor_tensor(
                    x1[:, xsl, :].rearrange("p (c x) y -> p c x y", c=2),
                    pv, -c2,
                    x0[:, xsl, :].rearrange("p (c x) y -> p c x y", c=2),
                    AX.mult, AX.add)
            x0 = x1

        nc.sync.dma_start(out_dram[:], x0[:])

    nc.compile()
    return nc


@lru_cache(maxsize=4)
def _compiled(scalars, T):
    return _build_nc(scalars, T)


def _make_in_maps(x, lambda_map, scalars, sig):
    bf = np.float16
    stats = _stationaries()
    a_, c2, th = scalars
    in_maps = []
    for rank in range(8):
        mbi, pos = rank // 4, rank % 4
        s = pos * NXS
        xs = slice(s, s + NXS)
        xn = np.ascontiguousarray(x[mbi, 0, xs]).astype(np.float32)
        lam = lambda_map[mbi].astype(np.float32) / np.float32(sig)
        # x-channel lambda on the 41-wide overlap slab [s-1, s+40)
        idx = [(s - 1 + j) % 160 for j in range(NXS + 1)]
        lx = lam[0][idx]
        nxt, prv = (pos + 1) % 4, (pos - 1) % 4
        mhi = np.zeros((128, 8), np.float32)
        mlo = np.zeros((128, 8), np.float32)
        mhi[:, 2 * nxt] = 1.0        # next's first plane -> halo_hi
        mlo[:, 2 * prv + 1] = 1.0    # prev's last plane  -> halo_lo
        m = dict(
            xb0=to_dev(xn).astype(bf),
            x00=to_dev(xn),
            mt0=to_dev(xn / np.float32(sig)).astype(bf),
            cxn=to_dev(np.float32(a_) * xn).astype(bf),
            lamx=to_dev(lx).astype(bf), nlamx=to_dev(-lx).astype(bf),
            lamy=to_dev(lam[1][xs]).astype(bf),
            nlamy=to_dev(-lam[1][xs]).astype(bf),
            lamt=to_dev(lam[2][xs]).astype(bf),
            nlamt=to_dev(-lam[2][xs]).astype(bf),
            mskhi=mhi, msklo=mlo,
        )
        m.update({k: v.astype(bf) for k, v in stats.items()})
        in_maps.append(m)
    return in_maps


def kernel(x, lambda_map, tau, sigma, theta):
    x = np.asarray(x, dtype=np.float32)
    lambda_map = np.asarray(lambda_map, dtype=np.float32)
    L = math.sqrt(13.0)
    sig = float(1.0 / (1.0 + math.exp(-float(np.asarray(sigma)[0])))) / L
    ta = float(1.0 / (1.0 + math.exp(-float(np.asarray(tau)[0])))) / L
    th = float(1.0 / (1.0 + math.exp(-float(np.asarray(theta)[0]))))
    a_ = 1.0 / (1.0 + sig)
    c2 = ta * sig
    scalars = tuple(float(np.float32(v)) for v in (a_, c2, th))

    nc = _compiled(scalars, T_ITERS)
    in_maps = _make_in_maps(x, lambda_map, scalars, sig)
    res = run_bass_kernel_spmd(nc, in_maps, core_ids=list(range(8)),
                               trace=TRACE)
    global _LAST_RESULTS
    _LAST_RESULTS = res

    out = np.zeros((2, 1, 160, 160, 32), np.float32)
    for rank in range(8):
        mbi, pos = rank // 4, rank % 4
        s = pos * NXS
        out[mbi, 0, s:s + NXS] = from_dev(res.results[rank]["out"])
    return out
